# revision 3
# baseline (speedup 1.0000x reference)
"""Trainium2 Bass kernel for CNNText: embedding gather + multi-width conv1d
+ bias/ReLU/max-pool + output matmul, data-parallel over batch on 8 NeuronCores.

Strategy per core (8 batch elements each):
  - Host: dedup words -> compact bf16 embedding table (<=32768 rows) so the
    int16-indexed transposing dma_gather can be used; pre-transpose conv
    filters/output layer on host (tiny).
  - Device: dma_gather(transpose=True) fuses the embedding gather with the
    [pos, D] -> [D, pos] transpose, landing emb_T tiles ready as matmul rhs.
    Conv(width w) = sum over kernel offsets i of emb_T shifted by i times the
    per-offset filter slice -> PSUM accumulation; ReLU(max+bias) after a
    free-dim max reduce; final [8,300]@[300,10] matmul on device.
"""
import numpy as np
import ml_dtypes
from contextlib import ExitStack

import concourse.tile as tile
from concourse import bacc, mybir
from concourse.bass_utils import run_bass_kernel_spmd

P = 128
SL = 512
D = 512
B = 64
NCORES = 8
NB = B // NCORES          # batch elems per core
LAYERNUM = 100
WIDTHS = [3, 4, 5]
NT = sum(WIDTHS)          # 12 (width, offset) filter tiles
KC = D // P               # 4 contraction chunks
VMAX = 32768              # compact table rows (max distinct words = 64*512)
DOUT = 10

BF16 = mybir.dt.bfloat16
F32 = mybir.dt.float32
I16 = mybir.dt.int16

_CACHE: dict = {}
LAST_RESULTS = None       # BassKernelResults of the most recent run (for profiling)


def _build():
    nc = bacc.Bacc("TRN2", target_bir_lowering=False, debug=False,
                   enable_asserts=True, num_devices=NCORES)

    table = nc.dram_tensor("table", [VMAX, D], BF16, kind="ExternalInput").ap()
    idx = nc.dram_tensor("idx", [P, NB * (SL // 16)], I16, kind="ExternalInput").ap()
    wts = nc.dram_tensor("wts", [P, KC * NT * LAYERNUM], BF16, kind="ExternalInput").ap()
    ol = nc.dram_tensor("ol", [LAYERNUM, 3 * DOUT], F32, kind="ExternalInput").ap()
    bias = nc.dram_tensor("bias", [LAYERNUM, 3], F32, kind="ExternalInput").ap()
    out = nc.dram_tensor("out", [NB, DOUT], F32, kind="ExternalOutput").ap()

    with tile.TileContext(nc) as tc:
        with ExitStack() as ctx:
            consts = ctx.enter_context(tc.tile_pool(name="consts", bufs=1))
            embp = ctx.enter_context(tc.tile_pool(name="emb", bufs=4))
            psump = ctx.enter_context(tc.tile_pool(name="psum", bufs=2, space="PSUM"))
            outp = ctx.enter_context(tc.tile_pool(name="outp", bufs=1))

            # idx first (gates gather 0), all on Pool: its DMA dispatch is
            # ~25ns vs 565ns+HWDGE on SP, and Pool is otherwise idle here.
            idx_t = consts.tile([P, NB, SL // 16], I16)
            nc.gpsimd.dma_start(idx_t[:], idx.rearrange("p (b s) -> p b s", b=NB))
            wt = consts.tile([P, KC, NT, LAYERNUM], BF16)
            nc.gpsimd.dma_start(wt[:], wts.rearrange("p (c t f) -> p c t f", c=KC, t=NT))
            ol_t = consts.tile([LAYERNUM, 3, DOUT], F32)
            nc.gpsimd.dma_start(ol_t[:], ol.rearrange("p (w o) -> p w o", w=3))
            bias_t = consts.tile([LAYERNUM, 3], F32)
            nc.gpsimd.dma_start(bias_t[:], bias)

            pooled = [outp.tile([LAYERNUM, NB], F32, tag=f"pool{wi}", name=f"pool{wi}")
                      for wi in range(3)]

            for b in range(NB):
                emb = embp.tile([P, KC, SL], BF16, tag="emb")
                nc.gpsimd.dma_gather(
                    emb[:], table[:], idx_t[:, b, :],
                    num_idxs=SL, num_idxs_reg=SL, elem_size=D,
                    transpose=True,
                )
                t0 = 0
                for wi, w in enumerate(WIDTHS):
                    ps = psump.tile([LAYERNUM, SL], F32, tag=f"ps{wi}")
                    for i in range(w):
                        for c in range(KC):
                            nc.tensor.matmul(
                                ps[:, 0:SL - i],
                                lhsT=wt[:, c, t0 + i, :],
                                rhs=emb[:, c, i:SL],
                                start=(i == 0 and c == 0),
                                stop=(i == w - 1 and c == KC - 1),
                            )
                    nc.vector.reduce_max(pooled[wi][:, b:b + 1], ps[:],
                                         axis=mybir.AxisListType.X)
                    t0 += w

            fin = psump.tile([NB, DOUT], F32, tag="fin")
            for wi in range(3):
                pr = outp.tile([LAYERNUM, NB], F32, tag=f"pr{wi}", name=f"pr{wi}")
                nc.scalar.activation(pr[:], pooled[wi][:],
                                     mybir.ActivationFunctionType.Relu,
                                     bias=bias_t[:, wi:wi + 1])
                nc.tensor.matmul(fin[:], lhsT=pr[:], rhs=ol_t[:, wi, :],
                                 start=(wi == 0), stop=(wi == 2))
            res = outp.tile([NB, DOUT], F32)
            nc.vector.tensor_copy(res[:], fin[:])
            nc.sync.dma_start(out, res[:])

    nc.compile()
    return nc


def _pack_idx(ridx):
    """[NB, SL] int16 -> [128, NB*SL/16]: position i -> partition i%16,
    col i//16, replicated over the 8 16-partition groups."""
    t16 = ridx.reshape(NB, SL // 16, 16).transpose(2, 0, 1)   # [16, NB, 32]
    return np.tile(t16, (8, 1, 1)).reshape(P, NB * (SL // 16)).copy()


def kernel(words, Embedding, outputlayer, filters_w3, bias_w3,
           filters_w4, bias_w4, filters_w5, bias_w5):
    global LAST_RESULTS
    words = np.asarray(words)
    Embedding = np.asarray(Embedding, dtype=np.float32)
    outputlayer = np.asarray(outputlayer, dtype=np.float32)
    filts = {3: np.asarray(filters_w3, dtype=np.float32),
             4: np.asarray(filters_w4, dtype=np.float32),
             5: np.asarray(filters_w5, dtype=np.float32)}
    biases = {3: np.asarray(bias_w3, dtype=np.float32),
              4: np.asarray(bias_w4, dtype=np.float32),
              5: np.asarray(bias_w5, dtype=np.float32)}

    # Dedup the vocabulary actually referenced so indices fit in int16 and
    # gather traffic shrinks (<= 32768 distinct of 50000 rows).
    uniq, inv = np.unique(words, return_inverse=True)
    table = np.zeros((VMAX, D), dtype=ml_dtypes.bfloat16)
    table[:len(uniq)] = Embedding[uniq].astype(ml_dtypes.bfloat16)
    inv = inv.reshape(B, SL).astype(np.int16)

    K_all = np.stack([filts[w].reshape(LAYERNUM, w, D)[:, i, :].T
                      for w in WIDTHS for i in range(w)])     # [12, 512, 100]
    wts = (K_all.reshape(NT, KC, P, LAYERNUM).transpose(2, 1, 0, 3)
           .reshape(P, KC * NT * LAYERNUM).astype(ml_dtypes.bfloat16))
    ol = (outputlayer.reshape(3, LAYERNUM, DOUT).transpose(1, 0, 2)
          .reshape(LAYERNUM, 3 * DOUT).copy())
    bias = np.stack([biases[w] for w in WIDTHS], axis=1).copy()

    in_maps = []
    for core in range(NCORES):
        ridx = inv[core * NB:(core + 1) * NB]
        in_maps.append({"table": table, "idx": _pack_idx(ridx),
                        "wts": wts, "ol": ol, "bias": bias})

    nc = _CACHE.get("nc")
    if nc is None:
        nc = _CACHE["nc"] = _build()

    res = run_bass_kernel_spmd(nc, in_maps, core_ids=list(range(NCORES)))
    LAST_RESULTS = res
    return np.concatenate([res.results[i]["out"] for i in range(NCORES)],
                          axis=0).astype(np.float32)


# revision 5
# speedup vs baseline: 1.0018x; 1.0018x over previous
"""Trainium2 Bass kernel for CNNText: embedding gather + multi-width conv1d
+ bias/ReLU/max-pool + output matmul, data-parallel over batch on 8 NeuronCores.

Strategy per core (8 batch elements each):
  - Host: dedup words -> compact bf16 embedding table (<=32768 rows) so the
    int16-indexed transposing dma_gather can be used; pre-transpose conv
    filters/output layer on host (tiny).
  - Device: dma_gather(transpose=True) fuses the embedding gather with the
    [pos, D] -> [D, pos] transpose, landing emb_T tiles ready as matmul rhs.
    Conv(width w) = sum over kernel offsets i of emb_T shifted by i times the
    per-offset filter slice -> PSUM accumulation; ReLU(max+bias) after a
    free-dim max reduce; final [8,300]@[300,10] matmul on device.
"""
import numpy as np
import ml_dtypes
from contextlib import ExitStack

import concourse.tile as tile
from concourse import bacc, mybir
from concourse import library_config
from concourse.bass_utils import run_bass_kernel_spmd

P = 128
SL = 512
D = 512
B = 64
NCORES = 8
NB = B // NCORES          # batch elems per core
LAYERNUM = 100
WIDTHS = [3, 4, 5]
NT = sum(WIDTHS)          # 12 (width, offset) filter tiles
KC = D // P               # 4 contraction chunks
VMAX = 32768              # compact table rows (max distinct words = 64*512)
DOUT = 10

BF16 = mybir.dt.bfloat16
F32 = mybir.dt.float32
I16 = mybir.dt.int16

_CACHE: dict = {}
LAST_RESULTS = None       # BassKernelResults of the most recent run (for profiling)


def _build():
    nc = bacc.Bacc("TRN2", target_bir_lowering=False, debug=False,
                   enable_asserts=True, num_devices=NCORES)

    table = nc.dram_tensor("table", [VMAX, D], BF16, kind="ExternalInput").ap()
    idx = nc.dram_tensor("idx", [P, NB * (SL // 16)], I16, kind="ExternalInput").ap()
    wts = nc.dram_tensor("wts", [P, KC * NT * LAYERNUM], BF16, kind="ExternalInput").ap()
    ol = nc.dram_tensor("ol", [LAYERNUM, 3 * DOUT], F32, kind="ExternalInput").ap()
    bias = nc.dram_tensor("bias", [LAYERNUM, 3], F32, kind="ExternalInput").ap()
    out = nc.dram_tensor("out", [NB, DOUT], F32, kind="ExternalOutput").ap()

    with tile.TileContext(nc) as tc:
        with ExitStack() as ctx:
            consts = ctx.enter_context(tc.tile_pool(name="consts", bufs=1))
            embp = ctx.enter_context(tc.tile_pool(name="emb", bufs=4))
            psump = ctx.enter_context(tc.tile_pool(name="psum", bufs=2, space="PSUM"))
            outp = ctx.enter_context(tc.tile_pool(name="outp", bufs=1))

            # The Q7 gather ucode library boot costs ~10us and is lazily
            # triggered by the first gather-class instruction. Front-load it
            # with a throwaway 16-index gather (indices memset to 0) so the
            # boot overlaps the constant DMAs instead of gating gather 0.
            dummy_idx = consts.tile([P, 1], I16)
            nc.gpsimd.memset(dummy_idx[:], 0)
            nc.gpsimd.load_library(library_config.mlp)
            dummy_out = consts.tile([P, D], BF16)
            nc.gpsimd.dma_gather(
                dummy_out[:].rearrange("p (a d) -> p a d", a=1), table[:],
                dummy_idx[:], num_idxs=16, num_idxs_reg=16, elem_size=D,
                transpose=False,
            )

            # Const loads on the SP HWDGE path (RTL descgen, no Q7): idx
            # first since it gates gather 0.
            idx_t = consts.tile([P, NB, SL // 16], I16)
            nc.sync.dma_start(idx_t[:], idx.rearrange("p (b s) -> p b s", b=NB))
            wt = consts.tile([P, KC, NT, LAYERNUM], BF16)
            nc.sync.dma_start(wt[:], wts.rearrange("p (c t f) -> p c t f", c=KC, t=NT))
            ol_t = consts.tile([LAYERNUM, 3, DOUT], F32)
            nc.sync.dma_start(ol_t[:], ol.rearrange("p (w o) -> p w o", w=3))
            bias_t = consts.tile([LAYERNUM, 3], F32)
            nc.sync.dma_start(bias_t[:], bias)

            pooled = [outp.tile([LAYERNUM, NB], F32, tag=f"pool{wi}", name=f"pool{wi}")
                      for wi in range(3)]

            for b in range(NB):
                emb = embp.tile([P, KC, SL], BF16, tag="emb")
                nc.gpsimd.dma_gather(
                    emb[:], table[:], idx_t[:, b, :],
                    num_idxs=SL, num_idxs_reg=SL, elem_size=D,
                    transpose=True,
                )
                t0 = 0
                for wi, w in enumerate(WIDTHS):
                    ps = psump.tile([LAYERNUM, SL], F32, tag=f"ps{wi}")
                    for i in range(w):
                        for c in range(KC):
                            nc.tensor.matmul(
                                ps[:, 0:SL - i],
                                lhsT=wt[:, c, t0 + i, :],
                                rhs=emb[:, c, i:SL],
                                start=(i == 0 and c == 0),
                                stop=(i == w - 1 and c == KC - 1),
                            )
                    nc.vector.reduce_max(pooled[wi][:, b:b + 1], ps[:],
                                         axis=mybir.AxisListType.X)
                    t0 += w

            fin = psump.tile([NB, DOUT], F32, tag="fin")
            for wi in range(3):
                pr = outp.tile([LAYERNUM, NB], F32, tag=f"pr{wi}", name=f"pr{wi}")
                nc.scalar.activation(pr[:], pooled[wi][:],
                                     mybir.ActivationFunctionType.Relu,
                                     bias=bias_t[:, wi:wi + 1])
                nc.tensor.matmul(fin[:], lhsT=pr[:], rhs=ol_t[:, wi, :],
                                 start=(wi == 0), stop=(wi == 2))
            res = outp.tile([NB, DOUT], F32)
            nc.vector.tensor_copy(res[:], fin[:])
            nc.sync.dma_start(out, res[:])

    nc.compile()
    return nc


def _pack_idx(ridx):
    """[NB, SL] int16 -> [128, NB*SL/16]: position i -> partition i%16,
    col i//16, replicated over the 8 16-partition groups."""
    t16 = ridx.reshape(NB, SL // 16, 16).transpose(2, 0, 1)   # [16, NB, 32]
    return np.tile(t16, (8, 1, 1)).reshape(P, NB * (SL // 16)).copy()


def kernel(words, Embedding, outputlayer, filters_w3, bias_w3,
           filters_w4, bias_w4, filters_w5, bias_w5):
    global LAST_RESULTS
    words = np.asarray(words)
    Embedding = np.asarray(Embedding, dtype=np.float32)
    outputlayer = np.asarray(outputlayer, dtype=np.float32)
    filts = {3: np.asarray(filters_w3, dtype=np.float32),
             4: np.asarray(filters_w4, dtype=np.float32),
             5: np.asarray(filters_w5, dtype=np.float32)}
    biases = {3: np.asarray(bias_w3, dtype=np.float32),
              4: np.asarray(bias_w4, dtype=np.float32),
              5: np.asarray(bias_w5, dtype=np.float32)}

    # Dedup the vocabulary actually referenced so indices fit in int16 and
    # gather traffic shrinks (<= 32768 distinct of 50000 rows).
    uniq, inv = np.unique(words, return_inverse=True)
    table = np.zeros((VMAX, D), dtype=ml_dtypes.bfloat16)
    table[:len(uniq)] = Embedding[uniq].astype(ml_dtypes.bfloat16)
    inv = inv.reshape(B, SL).astype(np.int16)

    K_all = np.stack([filts[w].reshape(LAYERNUM, w, D)[:, i, :].T
                      for w in WIDTHS for i in range(w)])     # [12, 512, 100]
    wts = (K_all.reshape(NT, KC, P, LAYERNUM).transpose(2, 1, 0, 3)
           .reshape(P, KC * NT * LAYERNUM).astype(ml_dtypes.bfloat16))
    ol = (outputlayer.reshape(3, LAYERNUM, DOUT).transpose(1, 0, 2)
          .reshape(LAYERNUM, 3 * DOUT).copy())
    bias = np.stack([biases[w] for w in WIDTHS], axis=1).copy()

    in_maps = []
    for core in range(NCORES):
        ridx = inv[core * NB:(core + 1) * NB]
        in_maps.append({"table": table, "idx": _pack_idx(ridx),
                        "wts": wts, "ol": ol, "bias": bias})

    nc = _CACHE.get("nc")
    if nc is None:
        nc = _CACHE["nc"] = _build()

    res = run_bass_kernel_spmd(nc, in_maps, core_ids=list(range(NCORES)))
    LAST_RESULTS = res
    return np.concatenate([res.results[i]["out"] for i in range(NCORES)],
                          axis=0).astype(np.float32)


# revision 10
# speedup vs baseline: 1.1329x; 1.1308x over previous
"""Trainium2 Bass kernel for CNNText: embedding gather + multi-width conv1d
+ bias/ReLU/max-pool + output matmul, data-parallel over batch on 8 NeuronCores.

Strategy per core (8 batch elements each):
  - Host: dedup words -> compact bf16 embedding table (<=32768 rows) so the
    int16-indexed transposing dma_gather can be used; pre-transpose conv
    filters/output layer on host (tiny).
  - Device: dma_gather(transpose=True) fuses the embedding gather with the
    [pos, D] -> [D, pos] transpose, landing emb_T tiles ready as matmul rhs.
    Conv(width w) = sum over kernel offsets i of emb_T shifted by i times the
    per-offset filter slice -> PSUM accumulation; ReLU(max+bias) after a
    free-dim max reduce; final [8,300]@[300,10] matmul on device.
"""
import numpy as np
import ml_dtypes
from contextlib import ExitStack

import concourse.tile as tile
from concourse import bacc, mybir
from concourse import library_config
from concourse.bass_utils import run_bass_kernel_spmd

P = 128
SL = 512
D = 512
B = 64
NCORES = 8
NB = B // NCORES          # batch elems per core
LAYERNUM = 100
WIDTHS = [3, 4, 5]
NT = sum(WIDTHS)          # 12 (width, offset) filter tiles
KC = D // P               # 4 contraction chunks
VMAX = 32768              # compact table rows (max distinct words = 64*512)
DOUT = 10

BF16 = mybir.dt.bfloat16
F32 = mybir.dt.float32
I16 = mybir.dt.int16

_CACHE: dict = {}
LAST_RESULTS = None       # BassKernelResults of the most recent run (for profiling)


def _build():
    nc = bacc.Bacc("TRN2", target_bir_lowering=False, debug=False,
                   enable_asserts=True, num_devices=NCORES)

    table = nc.dram_tensor("table", [VMAX, D], BF16, kind="ExternalInput").ap()
    idx = nc.dram_tensor("idx", [P, NB * (SL // 16)], I16, kind="ExternalInput").ap()
    # Host-pregathered emb_T for batch elems 0 and 1: plain DMA loads that
    # bypass the ~12us Q7 gather-library boot at kernel start.
    emb01 = nc.dram_tensor("emb01", [P, 2 * KC * SL], BF16, kind="ExternalInput").ap()
    wts = nc.dram_tensor("wts", [P, KC * NT * LAYERNUM], BF16, kind="ExternalInput").ap()
    ol = nc.dram_tensor("ol", [LAYERNUM, 3 * DOUT], F32, kind="ExternalInput").ap()
    bias = nc.dram_tensor("bias", [LAYERNUM, 3], F32, kind="ExternalInput").ap()
    out = nc.dram_tensor("out", [NB, DOUT], F32, kind="ExternalOutput").ap()

    with tile.TileContext(nc) as tc:
        with ExitStack() as ctx:
            consts = ctx.enter_context(tc.tile_pool(name="consts", bufs=1))
            embp = ctx.enter_context(tc.tile_pool(name="emb", bufs=4))
            psump = ctx.enter_context(tc.tile_pool(name="psum", bufs=2, space="PSUM"))
            outp = ctx.enter_context(tc.tile_pool(name="outp", bufs=1))

            # Kick the ~12us Q7 gather-library boot immediately; it overlaps
            # the b0/b1 compute whose embeddings arrive via plain DMA below.
            nc.gpsimd.load_library(library_config.mlp)

            # Const loads spread across the HWDGE engines (RTL descgen, no
            # Q7). wt alone on SP: it gates the first LDWEIGHTS.
            wt = consts.tile([P, KC, NT, LAYERNUM], BF16)
            nc.sync.dma_start(wt[:], wts.rearrange("p (c t f) -> p c t f", c=KC, t=NT))
            idx_t = consts.tile([P, NB, SL // 16], I16)
            nc.sync.dma_start(idx_t[:], idx.rearrange("p (b s) -> p b s", b=NB))
            ol_t = consts.tile([LAYERNUM, 3, DOUT], F32)
            nc.sync.dma_start(ol_t[:], ol.rearrange("p (w o) -> p w o", w=3))
            bias_t = consts.tile([LAYERNUM, 3], F32)
            nc.sync.dma_start(bias_t[:], bias)

            pooled = [outp.tile([LAYERNUM, NB], F32, tag=f"pool{wi}", name=f"pool{wi}")
                      for wi in range(3)]

            for b in range(NB):
                emb = embp.tile([P, KC, SL], BF16, tag="emb")
                if b < 2:
                    nc.scalar.dma_start(
                        emb[:],
                        emb01.rearrange("p (b c s) -> p b c s", b=2, c=KC)[:, b])
                else:
                    nc.gpsimd.dma_gather(
                        emb[:], table[:], idx_t[:, b, :],
                        num_idxs=SL, num_idxs_reg=SL, elem_size=D,
                        transpose=True,
                    )
                t0 = 0
                for wi, w in enumerate(WIDTHS):
                    ps = psump.tile([LAYERNUM, SL], F32, tag=f"ps{wi}")
                    for i in range(w):
                        for c in range(KC):
                            nc.tensor.matmul(
                                ps[:, 0:SL - i],
                                lhsT=wt[:, c, t0 + i, :],
                                rhs=emb[:, c, i:SL],
                                start=(i == 0 and c == 0),
                                stop=(i == w - 1 and c == KC - 1),
                            )
                    nc.vector.reduce_max(pooled[wi][:, b:b + 1], ps[:],
                                         axis=mybir.AxisListType.X)
                    t0 += w

            fin = psump.tile([NB, DOUT], F32, tag="fin")
            for wi in range(3):
                pr = outp.tile([LAYERNUM, NB], F32, tag=f"pr{wi}", name=f"pr{wi}")
                nc.scalar.activation(pr[:], pooled[wi][:],
                                     mybir.ActivationFunctionType.Relu,
                                     bias=bias_t[:, wi:wi + 1])
                nc.tensor.matmul(fin[:], lhsT=pr[:], rhs=ol_t[:, wi, :],
                                 start=(wi == 0), stop=(wi == 2))
            res = outp.tile([NB, DOUT], F32)
            nc.vector.tensor_copy(res[:], fin[:])
            nc.sync.dma_start(out, res[:])

    nc.compile()
    return nc


def _pack_idx(ridx):
    """[NB, SL] int16 -> [128, NB*SL/16]: position i -> partition i%16,
    col i//16, replicated over the 8 16-partition groups."""
    t16 = ridx.reshape(NB, SL // 16, 16).transpose(2, 0, 1)   # [16, NB, 32]
    return np.tile(t16, (8, 1, 1)).reshape(P, NB * (SL // 16)).copy()


def kernel(words, Embedding, outputlayer, filters_w3, bias_w3,
           filters_w4, bias_w4, filters_w5, bias_w5):
    global LAST_RESULTS
    words = np.asarray(words)
    Embedding = np.asarray(Embedding, dtype=np.float32)
    outputlayer = np.asarray(outputlayer, dtype=np.float32)
    filts = {3: np.asarray(filters_w3, dtype=np.float32),
             4: np.asarray(filters_w4, dtype=np.float32),
             5: np.asarray(filters_w5, dtype=np.float32)}
    biases = {3: np.asarray(bias_w3, dtype=np.float32),
              4: np.asarray(bias_w4, dtype=np.float32),
              5: np.asarray(bias_w5, dtype=np.float32)}

    # Dedup the vocabulary actually referenced so indices fit in int16 and
    # gather traffic shrinks (<= 32768 distinct of 50000 rows).
    uniq, inv = np.unique(words, return_inverse=True)
    table = np.zeros((VMAX, D), dtype=ml_dtypes.bfloat16)
    table[:len(uniq)] = Embedding[uniq].astype(ml_dtypes.bfloat16)
    inv = inv.reshape(B, SL).astype(np.int16)

    K_all = np.stack([filts[w].reshape(LAYERNUM, w, D)[:, i, :].T
                      for w in WIDTHS for i in range(w)])     # [12, 512, 100]
    wts = (K_all.reshape(NT, KC, P, LAYERNUM).transpose(2, 1, 0, 3)
           .reshape(P, KC * NT * LAYERNUM).astype(ml_dtypes.bfloat16))
    ol = (outputlayer.reshape(3, LAYERNUM, DOUT).transpose(1, 0, 2)
          .reshape(LAYERNUM, 3 * DOUT).copy())
    bias = np.stack([biases[w] for w in WIDTHS], axis=1).copy()

    in_maps = []
    for core in range(NCORES):
        ridx = inv[core * NB:(core + 1) * NB]
        # host gather+transpose of batch elems 0,1: [2, SL, D] rows ->
        # [P, 2, KC, SL] with d = c*128 + p
        g = table[ridx[:2]]                                   # [2, SL, D] bf16
        e01 = (g.reshape(2, SL, KC, P).transpose(3, 0, 2, 1)
               .reshape(P, 2 * KC * SL).copy())
        in_maps.append({"table": table, "idx": _pack_idx(ridx), "emb01": e01,
                        "wts": wts, "ol": ol, "bias": bias})

    nc = _CACHE.get("nc")
    if nc is None:
        nc = _CACHE["nc"] = _build()

    res = run_bass_kernel_spmd(nc, in_maps, core_ids=list(range(NCORES)))
    LAST_RESULTS = res
    return np.concatenate([res.results[i]["out"] for i in range(NCORES)],
                          axis=0).astype(np.float32)


# revision 13
# speedup vs baseline: 1.1615x; 1.0253x over previous
"""Trainium2 Bass kernel for CNNText: embedding gather + multi-width conv1d
+ bias/ReLU/max-pool + output matmul, data-parallel over batch on 8 NeuronCores.

Strategy per core (8 batch elements each):
  - Host: dedup words -> compact bf16 embedding table (<=32768 rows) so the
    int16-indexed transposing dma_gather can be used; pre-transpose conv
    filters/output layer on host (tiny).
  - Device: dma_gather(transpose=True) fuses the embedding gather with the
    [pos, D] -> [D, pos] transpose, landing emb_T tiles ready as matmul rhs.
    Conv(width w) = sum over kernel offsets i of emb_T shifted by i times the
    per-offset filter slice -> PSUM accumulation; ReLU(max+bias) after a
    free-dim max reduce; final [8,300]@[300,10] matmul on device.
"""
import numpy as np
import ml_dtypes
from contextlib import ExitStack

import concourse.tile as tile
from concourse import bacc, mybir
from concourse import library_config
from concourse.bass_utils import run_bass_kernel_spmd

P = 128
SL = 512
D = 512
B = 64
NCORES = 8
NB = B // NCORES          # batch elems per core
LAYERNUM = 100
WIDTHS = [3, 4, 5]
NT = sum(WIDTHS)          # 12 (width, offset) filter tiles
KC = D // P               # 4 contraction chunks
VMAX = 32768              # compact table rows (max distinct words = 64*512)
DOUT = 10

BF16 = mybir.dt.bfloat16
F32 = mybir.dt.float32
I16 = mybir.dt.int16

_CACHE: dict = {}
LAST_RESULTS = None       # BassKernelResults of the most recent run (for profiling)


def _build():
    nc = bacc.Bacc("TRN2", target_bir_lowering=False, debug=False,
                   enable_asserts=True, num_devices=NCORES)

    table = nc.dram_tensor("table", [VMAX, D], BF16, kind="ExternalInput").ap()
    idx = nc.dram_tensor("idx", [P, NB * (SL // 16)], I16, kind="ExternalInput").ap()
    # Host-pregathered emb_T for batch elems 0 and 1: plain DMA loads that
    # bypass the ~12us Q7 gather-library boot at kernel start.
    emb01 = nc.dram_tensor("emb01", [P, 2 * KC * SL], BF16, kind="ExternalInput").ap()
    wts = nc.dram_tensor("wts", [P, KC * NT * LAYERNUM], BF16, kind="ExternalInput").ap()
    ol = nc.dram_tensor("ol", [LAYERNUM, 3 * DOUT], F32, kind="ExternalInput").ap()
    bias = nc.dram_tensor("bias", [LAYERNUM, 3], F32, kind="ExternalInput").ap()
    out = nc.dram_tensor("out", [NB, DOUT], F32, kind="ExternalOutput").ap()

    with tile.TileContext(nc) as tc:
        with ExitStack() as ctx:
            consts = ctx.enter_context(tc.tile_pool(name="consts", bufs=1))
            embp = ctx.enter_context(tc.tile_pool(name="emb", bufs=4))
            psump = ctx.enter_context(tc.tile_pool(name="psum", bufs=2, space="PSUM"))
            outp = ctx.enter_context(tc.tile_pool(name="outp", bufs=1))

            # Small consts on Pool's SWDGE before the library reload (Pool is
            # otherwise idle until the first device gather at ~20us).
            idx_t = consts.tile([P, NB, SL // 16], I16)
            nc.gpsimd.dma_start(idx_t[:], idx.rearrange("p (b s) -> p b s", b=NB))
            ol_t = consts.tile([LAYERNUM, 3, DOUT], F32)
            nc.gpsimd.dma_start(ol_t[:], ol.rearrange("p (w o) -> p w o", w=3))
            bias_t = consts.tile([LAYERNUM, 3], F32)
            nc.gpsimd.dma_start(bias_t[:], bias)
            # Kick the ~12us Q7 gather-library boot; it overlaps the b0/b1
            # compute whose embeddings arrive via plain DMA below.
            nc.gpsimd.load_library(library_config.mlp)

            # Weights split per contraction chunk across both HWDGE engines so
            # the first matmuls start as soon as their chunk lands.
            wts_v = wts.rearrange("p (c t f) -> p c t f", c=KC, t=NT)
            emb01_v = emb01.rearrange("p (b c s) -> p b c s", b=2, c=KC)
            wt_c = []
            for c in range(KC):
                w_tile = consts.tile([P, NT, LAYERNUM], BF16, tag=f"wt{c}",
                                     name=f"wt{c}")
                wt_c.append(w_tile)
            nc.sync.dma_start(wt_c[0][:], wts_v[:, 0])
            emb_01 = [embp.tile([P, KC, SL], BF16, tag="emb", name=f"emb_b{b}")
                      for b in range(2)]
            nc.scalar.dma_start(emb_01[0][:], emb01_v[:, 0])
            nc.sync.dma_start(wt_c[1][:], wts_v[:, 1])
            nc.scalar.dma_start(wt_c[3][:], wts_v[:, 3])
            nc.sync.dma_start(wt_c[2][:], wts_v[:, 2])
            nc.scalar.dma_start(emb_01[1][:], emb01_v[:, 1])

            pooled = [outp.tile([LAYERNUM, NB], F32, tag=f"pool{wi}", name=f"pool{wi}")
                      for wi in range(3)]

            for b in range(NB):
                if b < 2:
                    emb = emb_01[b]
                else:
                    emb = embp.tile([P, KC, SL], BF16, tag="emb")
                    nc.gpsimd.dma_gather(
                        emb[:], table[:], idx_t[:, b, :],
                        num_idxs=SL, num_idxs_reg=SL, elem_size=D,
                        transpose=True,
                    )
                t0 = 0
                for wi, w in enumerate(WIDTHS):
                    ps = psump.tile([LAYERNUM, SL], F32, tag=f"ps{wi}")
                    for i in range(w):
                        for c in range(KC):
                            nc.tensor.matmul(
                                ps[:, 0:SL - i],
                                lhsT=wt_c[c][:, t0 + i, :],
                                rhs=emb[:, c, i:SL],
                                start=(i == 0 and c == 0),
                                stop=(i == w - 1 and c == KC - 1),
                            )
                    nc.vector.reduce_max(pooled[wi][:, b:b + 1], ps[:],
                                         axis=mybir.AxisListType.X)
                    t0 += w

            fin = psump.tile([NB, DOUT], F32, tag="fin")
            for wi in range(3):
                pr = outp.tile([LAYERNUM, NB], F32, tag=f"pr{wi}", name=f"pr{wi}")
                nc.scalar.activation(pr[:], pooled[wi][:],
                                     mybir.ActivationFunctionType.Relu,
                                     bias=bias_t[:, wi:wi + 1])
                nc.tensor.matmul(fin[:], lhsT=pr[:], rhs=ol_t[:, wi, :],
                                 start=(wi == 0), stop=(wi == 2))
            res = outp.tile([NB, DOUT], F32)
            nc.vector.tensor_copy(res[:], fin[:])
            nc.sync.dma_start(out, res[:])

    nc.compile()
    return nc


def _pack_idx(ridx):
    """[NB, SL] int16 -> [128, NB*SL/16]: position i -> partition i%16,
    col i//16, replicated over the 8 16-partition groups."""
    t16 = ridx.reshape(NB, SL // 16, 16).transpose(2, 0, 1)   # [16, NB, 32]
    return np.tile(t16, (8, 1, 1)).reshape(P, NB * (SL // 16)).copy()


def kernel(words, Embedding, outputlayer, filters_w3, bias_w3,
           filters_w4, bias_w4, filters_w5, bias_w5):
    global LAST_RESULTS
    words = np.asarray(words)
    Embedding = np.asarray(Embedding, dtype=np.float32)
    outputlayer = np.asarray(outputlayer, dtype=np.float32)
    filts = {3: np.asarray(filters_w3, dtype=np.float32),
             4: np.asarray(filters_w4, dtype=np.float32),
             5: np.asarray(filters_w5, dtype=np.float32)}
    biases = {3: np.asarray(bias_w3, dtype=np.float32),
              4: np.asarray(bias_w4, dtype=np.float32),
              5: np.asarray(bias_w5, dtype=np.float32)}

    # Dedup the vocabulary actually referenced so indices fit in int16 and
    # gather traffic shrinks (<= 32768 distinct of 50000 rows).
    uniq, inv = np.unique(words, return_inverse=True)
    table = np.zeros((VMAX, D), dtype=ml_dtypes.bfloat16)
    table[:len(uniq)] = Embedding[uniq].astype(ml_dtypes.bfloat16)
    inv = inv.reshape(B, SL).astype(np.int16)

    K_all = np.stack([filts[w].reshape(LAYERNUM, w, D)[:, i, :].T
                      for w in WIDTHS for i in range(w)])     # [12, 512, 100]
    wts = (K_all.reshape(NT, KC, P, LAYERNUM).transpose(2, 1, 0, 3)
           .reshape(P, KC * NT * LAYERNUM).astype(ml_dtypes.bfloat16))
    ol = (outputlayer.reshape(3, LAYERNUM, DOUT).transpose(1, 0, 2)
          .reshape(LAYERNUM, 3 * DOUT).copy())
    bias = np.stack([biases[w] for w in WIDTHS], axis=1).copy()

    in_maps = []
    for core in range(NCORES):
        ridx = inv[core * NB:(core + 1) * NB]
        # host gather+transpose of batch elems 0,1: [2, SL, D] rows ->
        # [P, 2, KC, SL] with d = c*128 + p
        g = table[ridx[:2]]                                   # [2, SL, D] bf16
        e01 = (g.reshape(2, SL, KC, P).transpose(3, 0, 2, 1)
               .reshape(P, 2 * KC * SL).copy())
        in_maps.append({"table": table, "idx": _pack_idx(ridx), "emb01": e01,
                        "wts": wts, "ol": ol, "bias": bias})

    nc = _CACHE.get("nc")
    if nc is None:
        nc = _CACHE["nc"] = _build()

    res = run_bass_kernel_spmd(nc, in_maps, core_ids=list(range(NCORES)))
    LAST_RESULTS = res
    return np.concatenate([res.results[i]["out"] for i in range(NCORES)],
                          axis=0).astype(np.float32)


# revision 14
# speedup vs baseline: 1.8777x; 1.6166x over previous
"""Trainium2 Bass kernel for CNNText: embedding gather + multi-width conv1d
+ bias/ReLU/max-pool + output matmul, data-parallel over batch on 8 NeuronCores.

Per core (8 batch elements):
  - Host: dedup words -> compact fp8(e4m3, x2^19) embedding table (<=32768
    rows, int16-indexable); filters pre-transposed/scaled (x2^10) to fp8 in
    the DoubleRow pair layout; scales are folded back out in the ReLU's
    `scale` operand (max-pool commutes with positive scaling).
  - Device: dma_gather(transpose=True) fuses gather + [pos,D]->[D,pos]
    transpose at 16-bit granularity, which for fp8 lands d-PAIRS per
    partition -- exactly the DoubleRow matmul operand layout (K=256 per
    chunk). Conv = PSUM-accumulated shifted matmuls; free-dim max reduce;
    relu(max*descale+bias); [8,300]@[300,10] on device.
  - Startup: batch elems 0,1 use host-pregathered emb (plain HWDGE DMA) to
    hide the ~12us Q7 gather-library boot; weights split per chunk across
    both HWDGE queues; small consts ride Pool's SWDGE before the reload.
"""
import numpy as np
import ml_dtypes
from contextlib import ExitStack

import concourse.tile as tile
from concourse import bacc, mybir
from concourse import library_config
from concourse.bass_utils import run_bass_kernel_spmd

P = 128
SL = 512
D = 512
B = 64
NCORES = 8
NB = B // NCORES
LAYERNUM = 100
WIDTHS = [3, 4, 5]
NT = sum(WIDTHS)          # 12 (width, offset) filter tiles
KC8 = 2                   # contraction chunks of 256 (d-pairs per partition)
VMAX = 32768
DOUT = 10
S_E, S_K = 2.0**19, 2.0**10   # fp8 pre-scales for embedding / filters

F8 = mybir.dt.float8e4
F32 = mybir.dt.float32
I16 = mybir.dt.int16
NPF8 = ml_dtypes.float8_e4m3

_CACHE: dict = {}
LAST_RESULTS = None


def _build():
    nc = bacc.Bacc("TRN2", target_bir_lowering=False, debug=False,
                   enable_asserts=True, num_devices=NCORES)

    table = nc.dram_tensor("table", [VMAX, D], F8, kind="ExternalInput").ap()
    idx = nc.dram_tensor("idx", [P, NB * (SL // 16)], I16, kind="ExternalInput").ap()
    emb01 = nc.dram_tensor("emb01", [P, 2 * KC8 * SL * 2], F8, kind="ExternalInput").ap()
    wts = nc.dram_tensor("wts", [P, KC8 * 2 * NT * LAYERNUM], F8, kind="ExternalInput").ap()
    ol = nc.dram_tensor("ol", [LAYERNUM, 3 * DOUT], F32, kind="ExternalInput").ap()
    bias = nc.dram_tensor("bias", [LAYERNUM, 3], F32, kind="ExternalInput").ap()
    out = nc.dram_tensor("out", [NB, DOUT], F32, kind="ExternalOutput").ap()

    with tile.TileContext(nc) as tc:
        with ExitStack() as ctx:
            consts = ctx.enter_context(tc.tile_pool(name="consts", bufs=1))
            embp = ctx.enter_context(tc.tile_pool(name="emb", bufs=4))
            psump = ctx.enter_context(tc.tile_pool(name="psum", bufs=2, space="PSUM"))
            outp = ctx.enter_context(tc.tile_pool(name="outp", bufs=1))

            # Small consts on Pool's SWDGE before the library reload (Pool is
            # otherwise idle until the first device gather).
            idx_t = consts.tile([P, NB, SL // 16], I16)
            nc.gpsimd.dma_start(idx_t[:], idx.rearrange("p (b s) -> p b s", b=NB))
            ol_t = consts.tile([LAYERNUM, 3, DOUT], F32)
            nc.gpsimd.dma_start(ol_t[:], ol.rearrange("p (w o) -> p w o", w=3))
            bias_t = consts.tile([LAYERNUM, 3], F32)
            nc.gpsimd.dma_start(bias_t[:], bias)
            # Kick the ~12us Q7 gather-library boot; overlaps b0/b1 compute.
            nc.gpsimd.load_library(library_config.mlp)

            wts_v = wts.rearrange("p (j e t f) -> p j e t f", j=KC8, e=2, t=NT)
            emb01_v = emb01.rearrange("p (b x) -> p b x", b=2)
            wt_c = []
            for j in range(KC8):
                w_tile = consts.tile([P, 2, NT, LAYERNUM], F8, tag=f"wt{j}",
                                     name=f"wt{j}")
                wt_c.append(w_tile)
            emb_01 = [embp.tile([P, KC8, SL, 2], F8, tag="emb", name=f"emb_b{b}")
                      for b in range(2)]
            nc.sync.dma_start(wt_c[0][:], wts_v[:, 0])
            nc.scalar.dma_start(
                emb_01[0][:].rearrange("p j s e -> p (j s e)"), emb01_v[:, 0])
            nc.sync.dma_start(wt_c[1][:], wts_v[:, 1])
            nc.scalar.dma_start(
                emb_01[1][:].rearrange("p j s e -> p (j s e)"), emb01_v[:, 1])

            pooled = [outp.tile([LAYERNUM, NB], F32, tag=f"pool{wi}", name=f"pool{wi}")
                      for wi in range(3)]

            for b in range(NB):
                if b < 2:
                    emb = emb_01[b]
                else:
                    emb = embp.tile([P, KC8, SL, 2], F8, tag="emb")
                    gview = (emb[:].rearrange("p j s e -> p (j s e)")
                             .rearrange("p (a b) -> p a b", b=SL))
                    nc.gpsimd.dma_gather(
                        gview, table[:], idx_t[:, b, :],
                        num_idxs=SL, num_idxs_reg=SL, elem_size=D,
                        transpose=True,
                    )
                t0 = 0
                for wi, w in enumerate(WIDTHS):
                    ps = psump.tile([LAYERNUM, SL], F32, tag=f"ps{wi}")
                    for i in range(w):
                        for j in range(KC8):
                            rhs = emb[:, j, i:SL, :].rearrange("p s e -> p e s")
                            nc.tensor.matmul(
                                ps[:, 0:SL - i],
                                lhsT=wt_c[j][:, :, t0 + i, :],
                                rhs=rhs,
                                start=(i == 0 and j == 0),
                                stop=(i == w - 1 and j == KC8 - 1),
                                perf_mode=mybir.MatmulPerfMode.DoubleRow,
                            )
                    nc.vector.reduce_max(pooled[wi][:, b:b + 1], ps[:],
                                         axis=mybir.AxisListType.X)
                    t0 += w

            fin = psump.tile([NB, DOUT], F32, tag="fin")
            for wi in range(3):
                pr = outp.tile([LAYERNUM, NB], F32, tag=f"pr{wi}", name=f"pr{wi}")
                nc.scalar.activation(pr[:], pooled[wi][:],
                                     mybir.ActivationFunctionType.Relu,
                                     bias=bias_t[:, wi:wi + 1],
                                     scale=float(1.0 / (S_E * S_K)))
                nc.tensor.matmul(fin[:], lhsT=pr[:], rhs=ol_t[:, wi, :],
                                 start=(wi == 0), stop=(wi == 2))
            res = outp.tile([NB, DOUT], F32)
            nc.vector.tensor_copy(res[:], fin[:])
            nc.sync.dma_start(out, res[:])

    nc.compile()
    return nc


def _pack_idx(ridx):
    """[NB, SL] int16 -> [128, NB*SL/16]: position i -> partition i%16,
    col i//16, replicated over the 8 16-partition groups."""
    t16 = ridx.reshape(NB, SL // 16, 16).transpose(2, 0, 1)
    return np.tile(t16, (8, 1, 1)).reshape(P, NB * (SL // 16)).copy()


def kernel(words, Embedding, outputlayer, filters_w3, bias_w3,
           filters_w4, bias_w4, filters_w5, bias_w5):
    global LAST_RESULTS
    words = np.asarray(words)
    Embedding = np.asarray(Embedding, dtype=np.float32)
    outputlayer = np.asarray(outputlayer, dtype=np.float32)
    filts = {3: np.asarray(filters_w3, dtype=np.float32),
             4: np.asarray(filters_w4, dtype=np.float32),
             5: np.asarray(filters_w5, dtype=np.float32)}
    biases = {3: np.asarray(bias_w3, dtype=np.float32),
              4: np.asarray(bias_w4, dtype=np.float32),
              5: np.asarray(bias_w5, dtype=np.float32)}

    # Dedup referenced vocab so indices fit int16 (<= 32768 distinct rows).
    uniq, inv = np.unique(words, return_inverse=True)
    table = np.zeros((VMAX, D), dtype=NPF8)
    table[:len(uniq)] = (Embedding[uniq] * np.float32(S_E)).astype(NPF8)
    inv = inv.reshape(B, SL).astype(np.int16)

    K_all = np.stack([filts[w].reshape(LAYERNUM, w, D)[:, i, :].T
                      for w in WIDTHS for i in range(w)])    # [12, 512, 100]
    K8 = np.clip(K_all * np.float32(S_K), -240, 240).astype(NPF8)
    # lhsT pair layout: [p, j, e, t, m] with d = 256*j + 2*p + e
    wts = (K8.reshape(NT, KC8, P, 2, LAYERNUM).transpose(2, 1, 3, 0, 4)
           .reshape(P, KC8 * 2 * NT * LAYERNUM).copy())
    ol = (outputlayer.reshape(3, LAYERNUM, DOUT).transpose(1, 0, 2)
          .reshape(LAYERNUM, 3 * DOUT).copy())
    bias = np.stack([biases[w] for w in WIDTHS], axis=1).copy()

    in_maps = []
    for core in range(NCORES):
        ridx = inv[core * NB:(core + 1) * NB]
        # host gather of batch elems 0,1 in the gather-transpose pair layout
        g = table[ridx[:2]]                                   # [2, SL, D] fp8
        e01 = (g.reshape(2, SL, KC8, P, 2).transpose(3, 0, 2, 1, 4)
               .reshape(P, 2 * KC8 * SL * 2).copy())
        in_maps.append({"table": table, "idx": _pack_idx(ridx), "emb01": e01,
                        "wts": wts, "ol": ol, "bias": bias})

    nc = _CACHE.get("nc")
    if nc is None:
        nc = _CACHE["nc"] = _build()

    res = run_bass_kernel_spmd(nc, in_maps, core_ids=list(range(NCORES)))
    LAST_RESULTS = res
    return np.concatenate([res.results[i]["out"] for i in range(NCORES)],
                          axis=0).astype(np.float32)
